# revision 17
# baseline (speedup 1.0000x reference)
"""Trainium2 Bass kernel for de-emphasis IIR: y[n] = x[n] + 0.97*y[n-1] along last axis.

Input: waveform (32, 2, 480000) f32 = 64 independent sequences of 480k samples.
Sharding: pure data parallel — 8 sequences per core across 8 NeuronCores.

Per core: the 8 sequences are split into 16 chunks each -> 128 partitions,
each owning a contiguous 30000-sample chunk. The recurrence y = c*y_prev + x
runs along the free dim with the hardware DVE scan (tensor_tensor_scan),
~2.125 ns/elem across 128 partitions. Chunk boundaries use an H-sample halo
warmup (0.97^720 ~ 3e-10, far below fp32 noise), so partitions are fully
independent and no cross-partition or cross-core communication is needed.

DMA structure (measured on HW): each HWDGE ring (SP=sync, ACT=scalar)
sustains ~205 GB/s; SDMA engines are latency-bound on pure reads
(~13 GB/s/engine) and only reach ~26 GB/s when read and write descriptors
interleave, capping mixed traffic at the ~370-395 GB/s HBM/NC limit.
So: loads ride SP, stores ride ACT, the first tiles are small so the
store stream starts ASAP (entering mixed mode early), and the last
stores split across both rings.
"""

import numpy as np

COEFF = 0.97

# Full-problem geometry (hardcoded; harness runs kernel() standalone).
N_CORES = 8
SEQ_TOTAL = 64  # 32*2
S = SEQ_TOTAL // N_CORES  # 8 sequences per core
N = 480000  # samples per sequence
K = 16  # chunks per sequence -> S*K = 128 partitions
H = 720  # halo (warmup) samples per chunk
# per-chunk tile widths; sum must be (N/K + H) = 30720. Small first tiles
# get the scan/store pipeline going early; small last tiles shrink the tail.
WIDTHS = (1280, 1280) + (2560,) * 10 + (1280, 1280)
BUFS = 8
NSS = 4

_BUILD_CACHE = {}


def build_deemph(S, N, K, H, widths, coeff=COEFF, bufs=8, nss=2):
    """Build the Bass program for one core: x[S,N] -> y[S,N]."""
    import concourse.bacc as bacc
    import concourse.mybir as mybir
    from concourse.mybir import AluOpType
    from concourse.tile import TileContext

    C = N // K  # chunk length
    P = S * K  # partitions
    assert N % K == 0, (N, K)
    widths = list(widths)
    assert sum(widths) == C + H, (sum(widths), C, H)
    T = len(widths)
    Wmax = max(widths)
    assert widths[0] > H
    nss = min(nss, T - 1)
    f32 = mybir.dt.float32

    # tile i covers per-chunk positions [starts[i]-H, starts[i]-H+widths[i])
    starts = []
    p = 0
    for w in widths:
        starts.append(p - H)
        p += w

    nc = bacc.Bacc(trn_type="TRN2", debug=False)
    x = nc.dram_tensor("x", [S, N], f32, kind="ExternalInput")
    y = nc.dram_tensor("y", [S, N], f32, kind="ExternalOutput")
    # [K, S, C] views: DMA pairing maps (k, s) -> partition k*S + s
    xt = x[:].rearrange("s (k j) -> s k j", k=K).transpose((1, 0, 2))
    yt = y[:].rearrange("s (k j) -> s k j", k=K).transpose((1, 0, 2))

    with TileContext(nc) as tc:
        with (
            tc.tile_pool(name="cpool", bufs=1) as cpool,
            tc.tile_pool(name="xpool", bufs=bufs) as xpool,
            tc.tile_pool(name="ypool", bufs=bufs) as ypool,
        ):
            ctile = cpool.tile([P, 1], f32)
            nc.vector.memset(ctile[:, :], coeff)
            half = K // 2
            # all loads first: each engine's emission order is its ring's
            # FIFO order, so deferred store-halves must not precede loads.
            xtiles = []
            for i, w in enumerate(widths):
                xtile = xpool.tile([P, Wmax], f32, tag="xt")
                if i == 0:
                    # chunk 0 of each seq (partitions 0..S): zero warmup
                    nc.vector.memset(xtile[0:S, 0:H], 0.0)
                    nc.sync.dma_start(xtile[0:S, H:w], x[:, 0 : w - H])
                    nc.scalar.dma_start(
                        xtile[S:P, 0:H], xt[0 : K - 1, :, C - H : C]
                    )
                    nc.sync.dma_start(
                        xtile[S : half * S, H:w], xt[1:half, :, 0 : w - H]
                    )
                    nc.scalar.dma_start(
                        xtile[half * S : P, H:w], xt[half:K, :, 0 : w - H]
                    )
                else:
                    lo = starts[i]
                    nc.sync.dma_start(xtile[:, 0:w], xt[:, :, lo : lo + w])
                xtiles.append(xtile)
            ytiles = []
            prev_y = None
            for i, w in enumerate(widths):
                ytile = ypool.tile([P, Wmax], f32, tag="yt")
                init = 0.0 if i == 0 else prev_y
                nc.vector.tensor_tensor_scan(
                    ytile[:, 0:w],
                    ctile[:, 0:1].broadcast_to((P, w)),
                    xtiles[i][:, 0:w],
                    init,
                    AluOpType.mult,
                    AluOpType.add,
                )
                prev_y = ytile[:, w - 1 : w]
                ytiles.append(ytile)
            for i, w in enumerate(widths):
                lo = starts[i]
                if i == 0:
                    nc.scalar.dma_start(yt[:, :, 0 : w - H], ytiles[i][:, H:w])
                elif i < T - nss:
                    nc.scalar.dma_start(yt[:, :, lo : lo + w], ytiles[i][:, 0:w])
                else:
                    nc.scalar.dma_start(
                        yt[0:half, :, lo : lo + w], ytiles[i][0 : half * S, 0:w]
                    )
            # SP-ring halves of the last nss stores, after all SP loads
            for i in range(T - nss, T):
                w, lo = widths[i], starts[i]
                if i == 0:
                    continue
                nc.sync.dma_start(
                    yt[half:K, :, lo : lo + w], ytiles[i][half * S : P, 0:w]
                )
    nc.compile()
    return nc


def _get_nc():
    key = (S, N, K, H, WIDTHS, BUFS, NSS)
    if key not in _BUILD_CACHE:
        _BUILD_CACHE[key] = build_deemph(S, N, K, H, WIDTHS, bufs=BUFS, nss=NSS)
    return _BUILD_CACHE[key]


def run(waveform: np.ndarray, **spmd_kwargs):
    """Run on 8 NeuronCores; returns (full_output, BassKernelResults)."""
    from concourse.bass_utils import run_bass_kernel_spmd

    waveform = np.asarray(waveform)
    orig_shape = waveform.shape
    x = np.ascontiguousarray(waveform.reshape(SEQ_TOTAL, N).astype(np.float32, copy=False))
    nc = _get_nc()
    in_maps = [{"x": x[S * c : S * (c + 1)]} for c in range(N_CORES)]
    res = run_bass_kernel_spmd(nc, in_maps, core_ids=list(range(N_CORES)), **spmd_kwargs)
    out = np.concatenate([r["y"] for r in res.results], axis=0)
    return out.reshape(orig_shape), res


def kernel(waveform: np.ndarray) -> np.ndarray:
    out, _ = run(waveform)
    return out


# revision 27
# speedup vs baseline: 1.2118x; 1.2118x over previous
"""Trainium2 Bass kernel for de-emphasis IIR: y[n] = x[n] + 0.97*y[n-1] along last axis.

Input: waveform (32, 2, 480000) f32 = 64 independent sequences of 480k samples.
Sharding: pure data parallel — 8 sequences per core across 8 NeuronCores.

Per core: the 8 sequences are split into 16 chunks each -> 128 partitions,
each owning a contiguous 30000-sample chunk. The recurrence y = c*y_prev + x
runs along the free dim with the hardware DVE scan (tensor_tensor_scan),
~2.125 ns/elem across 128 partitions. Chunk boundaries use an H-sample halo
warmup (0.97^720 ~ 3e-10, far below fp32 noise), so partitions are fully
independent and no cross-partition or cross-core communication is needed.

DMA structure (measured on HW): each HWDGE ring (SP=sync, ACT=scalar)
sustains ~205 GB/s; SDMA engines are latency-bound on pure reads
(~13 GB/s/engine) and only reach ~26 GB/s when read and write descriptors
interleave, capping mixed traffic at the ~370-395 GB/s HBM/NC limit.
So: loads ride SP, stores ride ACT, the first tiles are small so the
store stream starts ASAP (entering mixed mode early), and the last
stores split across both rings.
"""

import numpy as np

COEFF = 0.97

# Full-problem geometry (hardcoded; harness runs kernel() standalone).
N_CORES = 8
SEQ_TOTAL = 64  # 32*2
S = SEQ_TOTAL // N_CORES  # 8 sequences per core
N = 480000  # samples per sequence
K = 16  # chunks per sequence -> S*K = 128 partitions
H = 720  # halo (warmup) samples per chunk
# per-chunk tile widths; sum must be (N/K + H) = 30720. Small first tiles
# get the scan/store pipeline going early; small last tiles shrink the tail.
WIDTHS = (1280, 1280) + (2560,) * 10 + (1280, 1280)
BUFS = 8
NSS = 2
RAW = True  # use the raw-bacc builder (no TileContext overhead)

_BUILD_CACHE = {}


def build_deemph(S, N, K, H, widths, coeff=COEFF, bufs=8, nss=2):
    """Build the Bass program for one core: x[S,N] -> y[S,N]."""
    import concourse.bacc as bacc
    import concourse.mybir as mybir
    from concourse.mybir import AluOpType
    from concourse.tile import TileContext

    C = N // K  # chunk length
    P = S * K  # partitions
    assert N % K == 0, (N, K)
    widths = list(widths)
    assert sum(widths) == C + H, (sum(widths), C, H)
    T = len(widths)
    Wmax = max(widths)
    assert widths[0] > H
    nss = min(nss, T - 1)
    f32 = mybir.dt.float32

    # tile i covers per-chunk positions [starts[i]-H, starts[i]-H+widths[i])
    starts = []
    p = 0
    for w in widths:
        starts.append(p - H)
        p += w

    nc = bacc.Bacc(trn_type="TRN2", debug=False)
    x = nc.dram_tensor("x", [S, N], f32, kind="ExternalInput")
    y = nc.dram_tensor("y", [S, N], f32, kind="ExternalOutput")
    # [K, S, C] views: DMA pairing maps (k, s) -> partition k*S + s
    xt = x[:].rearrange("s (k j) -> s k j", k=K).transpose((1, 0, 2))
    yt = y[:].rearrange("s (k j) -> s k j", k=K).transpose((1, 0, 2))

    with TileContext(nc) as tc:
        with (
            tc.tile_pool(name="cpool", bufs=1) as cpool,
            tc.tile_pool(name="xpool", bufs=bufs) as xpool,
            tc.tile_pool(name="ypool", bufs=bufs) as ypool,
        ):
            ctile = cpool.tile([P, 1], f32)
            nc.vector.memset(ctile[:, :], coeff)
            half = K // 2
            # all loads first: each engine's emission order is its ring's
            # FIFO order, so deferred store-halves must not precede loads.
            xtiles = []
            for i, w in enumerate(widths):
                xtile = xpool.tile([P, Wmax], f32, tag="xt")
                if i == 0:
                    # chunk 0 of each seq (partitions 0..S): zero warmup
                    nc.vector.memset(xtile[0:S, 0:H], 0.0)
                    nc.sync.dma_start(xtile[0:S, H:w], x[:, 0 : w - H])
                    nc.scalar.dma_start(
                        xtile[S:P, 0:H], xt[0 : K - 1, :, C - H : C]
                    )
                    nc.sync.dma_start(
                        xtile[S : half * S, H:w], xt[1:half, :, 0 : w - H]
                    )
                    nc.scalar.dma_start(
                        xtile[half * S : P, H:w], xt[half:K, :, 0 : w - H]
                    )
                else:
                    lo = starts[i]
                    nc.sync.dma_start(xtile[:, 0:w], xt[:, :, lo : lo + w])
                xtiles.append(xtile)
            ytiles = []
            prev_y = None
            for i, w in enumerate(widths):
                ytile = ypool.tile([P, Wmax], f32, tag="yt")
                init = 0.0 if i == 0 else prev_y
                nc.vector.tensor_tensor_scan(
                    ytile[:, 0:w],
                    ctile[:, 0:1].broadcast_to((P, w)),
                    xtiles[i][:, 0:w],
                    init,
                    AluOpType.mult,
                    AluOpType.add,
                )
                prev_y = ytile[:, w - 1 : w]
                ytiles.append(ytile)
            for i, w in enumerate(widths):
                lo = starts[i]
                if i == 0:
                    nc.scalar.dma_start(yt[:, :, 0 : w - H], ytiles[i][:, H:w])
                elif i < T - nss:
                    nc.scalar.dma_start(yt[:, :, lo : lo + w], ytiles[i][:, 0:w])
                else:
                    nc.scalar.dma_start(
                        yt[0:half, :, lo : lo + w], ytiles[i][0 : half * S, 0:w]
                    )
            # SP-ring halves of the last nss stores, after all SP loads
            for i in range(T - nss, T):
                w, lo = widths[i], starts[i]
                if i == 0:
                    continue
                nc.sync.dma_start(
                    yt[half:K, :, lo : lo + w], ytiles[i][half * S : P, 0:w]
                )
    nc.compile()
    return nc


def build_deemph_raw(S, N, K, H, widths, coeff=COEFF, bufs=8, nss=2):
    """Raw bacc builder: same pipeline as build_deemph but with hand-rolled
    semaphores instead of TileContext — saves Tile's entry barrier and
    ~12us exit drain/EVSEM butterfly.

    Engines: sync = load ring (+ final store halves), scalar = store ring
    (+ tile-0 load halves), vector = memsets + scans.
    Counting semaphores: load_sem/store_sem (+16 per DMA), scan_sem (+1).
    """
    import concourse.bacc as bacc
    import concourse.mybir as mybir
    from concourse.mybir import AluOpType

    C = N // K
    P = S * K
    assert N % K == 0
    widths = list(widths)
    assert sum(widths) == C + H
    T = len(widths)
    Wmax = max(widths)
    assert widths[0] > H
    nss = min(nss, T - 1)
    f32 = mybir.dt.float32

    starts = []
    p = 0
    for w in widths:
        starts.append(p - H)
        p += w

    assert nss <= bufs  # y-slot waits stay within ACT-only store range

    nc = bacc.Bacc(trn_type="TRN2", debug=False)
    x = nc.dram_tensor("x", [S, N], f32, kind="ExternalInput")
    y = nc.dram_tensor("y", [S, N], f32, kind="ExternalOutput")
    xt = x[:].rearrange("s (k j) -> s k j", k=K).transpose((1, 0, 2))
    yt = y[:].rearrange("s (k j) -> s k j", k=K).transpose((1, 0, 2))

    half = K // 2
    xbuf = nc.alloc_sbuf_tensor("xbuf", [P, bufs * Wmax], f32)
    ybuf = nc.alloc_sbuf_tensor("ybuf", [P, bufs * Wmax], f32)
    cbuf = nc.alloc_sbuf_tensor("cbuf", [P, 1], f32)

    def xsl(i):
        o = (i % bufs) * Wmax
        return xbuf[:, o : o + widths[i]]

    def ysl(i):
        o = (i % bufs) * Wmax
        return ybuf[:, o : o + widths[i]]

    # per-tile semaphores: every wait is at an "all writers done" value,
    # which is the only ordering the DMA completion model guarantees
    xsem = [nc.alloc_semaphore(f"xsem{i}") for i in range(T)]
    ysem = [nc.alloc_semaphore(f"ysem{i}") for i in range(T)]
    scan_sem = nc.alloc_semaphore("scan_sem")
    init_sem = nc.alloc_semaphore("init_sem")
    n_load = [4] + [1] * (T - 1)  # DMAs per x tile (tile 0: 4)
    n_store = [1 if i < T - nss else 2 for i in range(T)]

    with nc.Block() as block:

        @block.sync
        def _(sync):
            for i, w in enumerate(widths):
                if i >= bufs:
                    sync.wait_ge(scan_sem, i - bufs + 1)
                xv = xsl(i)
                if i == 0:
                    sync.dma_start(xv[0:S, H:w], x[:, 0 : w - H]).then_inc(
                        xsem[0], 16
                    )
                    sync.dma_start(
                        xv[S : half * S, H:w], xt[1:half, :, 0 : w - H]
                    ).then_inc(xsem[0], 16)
                else:
                    lo = starts[i]
                    sync.dma_start(xv[:, 0:w], xt[:, :, lo : lo + w]).then_inc(
                        xsem[i], 16
                    )
            for i in range(T - nss, T):
                w, lo = widths[i], starts[i]
                sync.wait_ge(scan_sem, i + 1)
                sync.dma_start(
                    yt[half:K, :, lo : lo + w], ysl(i)[half * S : P, 0:w]
                ).then_inc(ysem[i], 16)
            for i in range(T):
                sync.wait_ge(ysem[i], 16 * n_store[i])

        @block.scalar
        def _(scalar):
            w = widths[0]
            xv = xsl(0)
            scalar.dma_start(
                xv[S:P, 0:H], xt[0 : K - 1, :, C - H : C]
            ).then_inc(xsem[0], 16)
            scalar.dma_start(
                xv[half * S : P, H:w], xt[half:K, :, 0 : w - H]
            ).then_inc(xsem[0], 16)
            for i, w in enumerate(widths):
                lo = starts[i]
                scalar.wait_ge(scan_sem, i + 1)
                if i == 0:
                    scalar.dma_start(
                        yt[:, :, 0 : w - H], ysl(0)[:, H:w]
                    ).then_inc(ysem[0], 16)
                elif i < T - nss:
                    scalar.dma_start(
                        yt[:, :, lo : lo + w], ysl(i)[:, 0:w]
                    ).then_inc(ysem[i], 16)
                else:
                    scalar.dma_start(
                        yt[0:half, :, lo : lo + w], ysl(i)[0 : half * S, 0:w]
                    ).then_inc(ysem[i], 16)
            for i in range(T):
                scalar.wait_ge(ysem[i], 16 * n_store[i])

        @block.vector
        def _(vector):
            vector.memset(cbuf[:, :], coeff).then_inc(init_sem, 1)
            vector.memset(xsl(0)[0:S, 0:H], 0.0).then_inc(init_sem, 1)
            prev = None
            for i, w in enumerate(widths):
                if i == 0:
                    vector.wait_ge(init_sem, 2)
                else:
                    # scan i reads scan i-1's last column (initial); the DVE
                    # pipe needs the @complete sem, program order isn't enough
                    vector.wait_ge(scan_sem, i)
                vector.wait_ge(xsem[i], 16 * n_load[i])
                if i >= bufs:
                    vector.wait_ge(ysem[i - bufs], 16 * n_store[i - bufs])
                yv = ysl(i)
                vector.tensor_tensor_scan(
                    yv[:, 0:w],
                    cbuf[:, 0:1].broadcast_to((P, w)),
                    xsl(i)[:, 0:w],
                    0.0 if prev is None else prev,
                    AluOpType.mult,
                    AluOpType.add,
                ).then_inc(scan_sem, 1)
                prev = yv[:, w - 1 : w]

    nc.compile()
    return nc


def _get_nc():
    key = (S, N, K, H, WIDTHS, BUFS, NSS, RAW)
    if key not in _BUILD_CACHE:
        builder = build_deemph_raw if RAW else build_deemph
        _BUILD_CACHE[key] = builder(S, N, K, H, WIDTHS, bufs=BUFS, nss=NSS)
    return _BUILD_CACHE[key]


def run(waveform: np.ndarray, **spmd_kwargs):
    """Run on 8 NeuronCores; returns (full_output, BassKernelResults)."""
    from concourse.bass_utils import run_bass_kernel_spmd

    waveform = np.asarray(waveform)
    orig_shape = waveform.shape
    x = np.ascontiguousarray(waveform.reshape(SEQ_TOTAL, N).astype(np.float32, copy=False))
    nc = _get_nc()
    in_maps = [{"x": x[S * c : S * (c + 1)]} for c in range(N_CORES)]
    res = run_bass_kernel_spmd(nc, in_maps, core_ids=list(range(N_CORES)), **spmd_kwargs)
    out = np.concatenate([r["y"] for r in res.results], axis=0)
    return out.reshape(orig_shape), res


def kernel(waveform: np.ndarray) -> np.ndarray:
    out, _ = run(waveform)
    return out
